# revision 1
# baseline (speedup 1.0000x reference)
"""AvgPool2d(64x64, stride 1) with replicate-padding back to (512, 512),
as a distributed Bass kernel on 8 TRN2 NeuronCores.

Input : x (8, 64, 512, 512) float32
Output: (8, 64, 512, 512) float32

Strategy (pure data parallel): one batch element per core. Per core the
pooling is a separable 64-wide box filter; both directions are computed
on the TensorEngine as matmuls against a banded 0/1-matrix `band` of
shape [512, 512] with band[h, i] = 1/64 iff clamp(i-31, 0, 448) <= h <
clamp(i-31, 0, 448) + 64 (the clamp folds the replicate-padding into the
matrix, and the 1/64 folds the averaging).

    V^T = (X^T @ band)        pass 1: vertical box mean, transposed
    O   = (V^T)^T @ band      pass 2: horizontal box mean, natural layout

Both passes put the *data* tile in the stationary (lhsT) operand and the
band in the moving operand, which avoids every transpose. Data is cast
to bf16 (rel. error ~3e-3, well within tolerance); accumulation is fp32
in PSUM.

DMA-descriptor tuning: SDMA engines run ~2x faster per byte on >=4KB
contiguous descriptors than on the 2KB (one image row) chunks a
block-partition layout yields. So:
  - pass 1 loads X with partition p holding DRAM-contiguous row pairs
    (2p, 2p+1) per image half -> 4KB f32 reads (cast to bf16 in the
    SWDGE DMA). The contraction dim is permuted identically in a second
    copy of the band, so the matmul is unchanged; the only cost is a
    wider nonzero column range per K-section (1150 vs 701 cols).
  - pass 2 permutes the output partition dim via strided lhsT column
    slices (i_out = 4p + t) so partition p holds 4 consecutive output
    rows -> 8KB contiguous HWDGE writes. Free: the moving-side column
    restriction only depends on the (unpermuted) contraction blocks.
"""

import numpy as np
import ml_dtypes

C, H, W = 64, 512, 512
P = 128
NKH = H // P  # 4 partition blocks
KERNEL = 64
OUT_VALID = H - KERNEL + 1  # 449
PT = (H - OUT_VALID) // 2  # 31 (left/top pad)

# Pass-2 matmul plan, one PSUM tile (N = j_out): the contraction runs
# over standard 128-row blocks kw of V^T; each instruction's column
# range is uniformly "first writer" or "accumulating" so per-element
# PSUM has_written semantics hold: (kw, lo, hi, start, stop).
MM_PLAN_BLOCK = [
    (0, 0, 159, True, False),
    (1, 96, 159, False, False),
    (1, 159, 287, False, False),
    (2, 224, 287, False, False),
    (2, 287, 415, False, False),
    (3, 352, 415, False, False),
    (3, 415, 512, False, True),
]
# Pass-1 plan: K-sections are (half, parity) combs — h = 256*hf + 2p + t
# with section s = 2*hf + t. Sections 0/1 cover h in [0,255] (nonzero
# i_out [0,287)), sections 2/3 cover [256,511] (nonzero i_out [224,512)).
MM_PLAN_HALFPAR = [
    (0, 0, 287, True, False),
    (1, 0, 287, False, False),
    (2, 224, 287, False, False),
    (2, 287, 512, False, False),
    (3, 224, 512, False, True),
]
# Pass-1 plan for the quad-comb layout (h = 4p + t): every section spans
# the full h range, so every matmul is full-width.
MM_PLAN_QUAD = [
    (0, 0, 512, True, False),
    (1, 0, 512, False, False),
    (2, 0, 512, False, False),
    (3, 0, 512, False, True),
]
# QUAD_IN=True: input loaded with partition p = 4 consecutive DRAM rows
# (8KB descriptors, ~25% faster SDMA) at the cost of full-width pass-1
# matmuls (8192 vs 4600 cycles/channel on the PE).
QUAD_IN = False
# IN_HWDGE_F32=True (implies QUAD_IN layout): load the input as pure f32
# over the SP HWDGE ring (8KB descriptors run ~26 GB/s/engine vs the
# SWDGE cast path's ~21.3 GB/s cap) and cast f32->bf16 on Vector/Scalar,
# which have headroom. Output moves to the ACT HWDGE ring.
IN_HWDGE_F32 = False
# HYBRID_QUAD: route 3 of every 8 channels through the ACT HWDGE ring as
# pure-f32 quad-layout loads (8KB descs, engine-side cast, full-width
# pass 1) and the rest through the SWDGE cast path — shrinks aggregate
# SDMA engine time while the extra PE work stays under the stream window.
# 0 disables (exactly the pure-SWDGE halfpar kernel).
HYBRID_QUAD = 0  # channels per group of 8 using the quad/HWDGE path (0 = proven best: pure SWDGE half-pair)


def make_band() -> np.ndarray:
    i = np.arange(H)
    ic = np.clip(i - PT, 0, OUT_VALID - 1)
    h = np.arange(H)
    band = (h[:, None] >= ic[None, :]) & (h[:, None] < ic[None, :] + KERNEL)
    return (band.astype(np.float32) / KERNEL).astype(ml_dtypes.bfloat16)


def build_avgpool(tc, x_ap, band_ap, out_ap, channels=C):
    import concourse.mybir as mybir

    nc = tc.nc
    f32 = mybir.dt.float32
    bf16 = mybir.dt.bfloat16

    with (
        tc.tile_pool(name="const", bufs=1) as const_pool,
        tc.tile_pool(name="work", bufs=4) as work,
        tc.tile_pool(name="vtps", bufs=4, space="PSUM") as vt_psum,
        tc.tile_pool(name="ops", bufs=4, space="PSUM") as o_psum,
    ):
        # band in standard block layout: [p, kw, i] = band[128*kw + p, i]
        band_t = const_pool.tile([P, NKH, H], bf16, tag="band")
        nc.sync.dma_start(band_t[:], band_ap.rearrange("(kh p) i -> p kh i", p=P))
        use_quad_all = QUAD_IN or IN_HWDGE_F32
        if use_quad_all or HYBRID_QUAD:
            # band in quad-comb layout: [p, t, i] = band[4*p + t, i]
            band_q = const_pool.tile([P, 4, H], bf16, tag="bandq")
            nc.sync.dma_start(
                band_q[:], band_ap.rearrange("(p four) i -> p four i", p=P)
            )
        if not use_quad_all:
            # band in (half, parity) comb layout: [p, 2*hf+t, i] = band[256*hf + 2*p + t, i]
            band_hp4 = const_pool.tile([P, 2, 2, H], bf16, tag="bandhp")
            nc.sync.dma_start(
                band_hp4[:],
                band_ap.rearrange("(half p two) i -> p half two i", p=P, two=2),
            )
            band_hp = band_hp4.rearrange("p a b i -> p (a b) i")

        for c in range(channels):
            quad_c = use_quad_all or (HYBRID_QUAD and c % 8 < HYBRID_QUAD)
            # X permuted so each partition reads DRAM-contiguous chunks
            # (8KB quad / 4KB half-pair), bf16-cast in the SWDGE DMA or
            # on Vector/Scalar (HWDGE f32 path).
            xb = work.tile([P, 4, W], bf16, tag="xb")
            if quad_c and (IN_HWDGE_F32 or HYBRID_QUAD):
                xf = work.tile([P, 4 * W], f32, tag="xf")
                in_dma = nc.scalar.dma_start if HYBRID_QUAD else nc.sync.dma_start
                in_dma(xf[:], x_ap[c].rearrange("(p four) w -> p (four w)", p=P))
                if c % 2 == 0:
                    nc.vector.tensor_copy(xb[:].rearrange("p a w -> p (a w)"), xf[:])
                else:
                    nc.scalar.copy(xb[:].rearrange("p a w -> p (a w)"), xf[:])
            elif quad_c:
                nc.gpsimd.dma_start(
                    xb[:].rearrange("p a w -> p (a w)"),
                    x_ap[c].rearrange("(p four) w -> p (four w)", p=P),
                )
            else:
                # Two independent half-image DMAs: pass-1 sections 0/1 only
                # read half 0, so their matmuls overlap half 1's transfer.
                for hf in range(2):
                    nc.gpsimd.dma_start(
                        xb[:, 2 * hf : 2 * hf + 2, :].rearrange("p a w -> p (a w)"),
                        x_ap[c][256 * hf : 256 * (hf + 1)].rearrange(
                            "(p two) w -> p (two w)", p=P, two=2
                        ),
                    )

            # pass 1: V^T[w, i] = sum_h X[h, w] * band[h, i]
            p1_plan = MM_PLAN_QUAD if quad_c else MM_PLAN_HALFPAR
            band_p1 = band_q if quad_c else band_hp
            vtb = work.tile([P, NKH, H], bf16, tag="vtb")
            for mw in range(NKH):
                vt_ps = vt_psum.tile([P, H], f32, tag="vt")
                for s, lo, hi, start, stop in p1_plan:
                    nc.tensor.matmul(
                        vt_ps[:, lo:hi],
                        xb[:, s, P * mw : P * (mw + 1)],
                        band_p1[:, s, lo:hi],
                        start=start,
                        stop=stop,
                    )
                nc.scalar.copy(vtb[:, mw, :], vt_ps[:])

            # pass 2: O[i, j] = sum_w V^T[w, i] * band[w, j], with the
            # output partition dim permuted (i_out = 4p + t) so partition
            # p accumulates 4 consecutive output rows.
            o_sb = work.tile([P, 4 * W], f32, tag="osb")
            for t in range(4):
                o_ps = o_psum.tile([P, W], f32, tag="o")
                for kw, lo, hi, start, stop in MM_PLAN_BLOCK:
                    nc.tensor.matmul(
                        o_ps[:, lo:hi],
                        vtb[:, kw, t:H:4],
                        band_t[:, kw, lo:hi],
                        start=start,
                        stop=stop,
                    )
                nc.vector.tensor_copy(o_sb[:, 512 * t : 512 * (t + 1)], o_ps[:])

            # partition p holds rows 4p..4p+3 -> one 8KB contiguous
            # chunk per partition; HWDGE (no gpsimd descriptor-generation
            # cost). With IN_HWDGE_F32 the input owns the SP ring, so the
            # output takes the ACT ring.
            out_dma = nc.scalar.dma_start if IN_HWDGE_F32 else nc.sync.dma_start
            out_dma(out_ap[c].rearrange("(p four) j -> p (four j)", p=P), o_sb[:])


def build_nc(channels=C):
    import concourse.mybir as mybir
    import concourse.tile as tile
    from concourse import bacc

    # Bacc (not raw Bass): its compile() runs generate_event_semaphores,
    # which splits multi-semaphore waits — walrus codegen allows at most
    # one wait command per DMA instruction.
    nc = bacc.Bacc()
    x = nc.dram_tensor("x", [channels, H, W], mybir.dt.float32, kind="ExternalInput")
    band = nc.dram_tensor("band", [H, W], mybir.dt.bfloat16, kind="ExternalInput")
    out = nc.dram_tensor("out", [channels, H, W], mybir.dt.float32, kind="ExternalOutput")
    with tile.TileContext(nc) as tc:
        build_avgpool(tc, x.ap(), band.ap(), out.ap(), channels)
    nc.compile()
    return nc


def _ensure_axon_ntff_hook():
    """If tracing is requested (BASS_TRACE) under axon, run_bass_kernel_spmd
    imports antenv.axon_hooks, which some agent images lack. Install the
    real hook if possible, else a stub that degrades tracing gracefully."""
    import sys
    import types

    try:
        import antenv.axon_hooks  # noqa: F401

        return
    except Exception:
        pass
    try:
        import antenv
    except Exception:
        return
    mod = types.ModuleType("antenv.axon_hooks")
    mod._hook = None
    mod.set_axon_ntff_profile_hook = lambda h: setattr(mod, "_hook", h)
    mod.get_axon_ntff_profile_hook = lambda: mod._hook
    sys.modules["antenv.axon_hooks"] = mod
    antenv.axon_hooks = mod
    try:
        from trn_agent_boot.trn_boot import _ntff_profile_via_ctypes

        hook = _ntff_profile_via_ctypes("/opt/axon/libaxon_pjrt.so")
        if hook is not None:
            mod.set_axon_ntff_profile_hook(hook)
    except Exception:
        pass


def kernel(x) -> np.ndarray:
    _ensure_axon_ntff_hook()
    from concourse.bass_utils import run_bass_kernel_spmd

    x = np.asarray(x, dtype=np.float32)
    assert x.shape == (8, C, H, W)
    nc = build_nc()
    band = make_band()
    in_maps = [
        {"x": np.ascontiguousarray(x[b]), "band": band} for b in range(x.shape[0])
    ]
    res = run_bass_kernel_spmd(nc, in_maps, core_ids=list(range(8)))
    return np.stack([r["out"] for r in res.results], axis=0)

